# revision 32
# baseline (speedup 1.0000x reference)
"""Trainium2 Bass kernel for nn_BiPixelMambaLayer.

Self-contained: takes the FULL unsharded inputs (as produced by the problem's
setup_inputs), shards the NB=100 pixel-shuffled sequences across 8 NeuronCores,
runs a Bass/Tile kernel per core, and reassembles the full output.

Per-core algorithm (S=14 sequence slots of length L=1024, d_model=96):
  LN -> fused in_proj+causal depthwise conv (4 shifted PE matmuls with
  tap-folded weights, PSUM accumulate) -> Silu (direct from PSUM)
  -> x_proj -> dt_proj -> Softplus (direct from PSUM)
  -> exact selective scan (chunked, carry columns, bf16 lattice, DVE
     tensor_tensor_scan over flattened (n, d12, t) runs with zero-dA
     boundary columns) -> C-contraction (tree reduce over n) -> gating
  -> out_proj -> +residual.

Scan layout: partition p = s*16 + d16 (8 seqs x 16), free = (n=16, d12=12, t),
with d = d16*12 + d12.  A(d, n) = -exp(A_log)[0, n] is constant across d
(S4D init); the exact per-n fp32 decay rates are baked in as ACT Exp scales.

Engine budget: the DVE scan (2.1 ns/elem, serial recurrence) is the hard
floor; everything else is pushed to PE (conv taps), ACT (exp/silu/softplus,
PSUM evacuation), GpSimd (small carry copies) or hoisted out of the loop
(pad/boundary memsets).
"""
import contextlib
import numpy as np
import ml_dtypes

import concourse.bass as bass
import concourse.tile as tile
from concourse import mybir
from concourse.bass_utils import run_bass_kernel_spmd

BF16 = mybir.dt.bfloat16
F32 = mybir.dt.float32
AF = mybir.ActivationFunctionType
OP = mybir.AluOpType

# ---------------- problem constants ----------------
D_MODEL = 96
D_STATE = 16      # n
D_CONV = 4
D_INNER = 192     # d
DT_RANK = 6
P_PIX = 10
LN_EPS = 1e-5
HW_ = 320
NH = HW_ // P_PIX           # 32
L_FULL = NH * NH            # 1024
NB = 100
NCORES = 8
D16 = 16
D12 = 12
SGRP = (8, 6)               # sequence groups over S=14 (partitions = s*16+d16)
PAD = D_CONV - 1            # 3 history columns per sequence


class Cfg:
    def __init__(self, L=L_FULL, T=64, S=14):
        assert L % T == 0
        self.L = L
        self.T = T
        self.NCH = L // T
        self.S = S
        self.TOK = S * L
        self.SH = S // 2            # 7 per split
        self.LP = L + PAD           # padded per-sequence stride in xn_T


# ---------------- device kernel ----------------

def build_kernel(nc, tc, cfg, a_vals, engines=None):
    """Emit the full per-core kernel into nc (inside TileContext tc).

    a_vals: 16 positive floats = exp(A_log)[0, :] (decay rate per state n).
    """
    eng = {"bbuild": "vector", "pmul": "vector", "tree": "vector",
           "scan": "vector"}
    if engines:
        eng.update(engines)
    T, NCH, S, TOK, Lc, SH, LP = (cfg.T, cfg.NCH, cfg.S, cfg.TOK, cfg.L,
                                  cfg.SH, cfg.LP)

    # ---- DRAM I/O ----
    xtok = nc.dram_tensor("xtok", [TOK, D_MODEL], F32, kind="ExternalInput").ap()
    x_T = nc.dram_tensor("x_T", [D_MODEL, TOK], F32, kind="ExternalInput").ap()
    dram = {}
    for s_ in ("f", "b"):
        for nm, shape, dt_ in (
                (f"w_u_{s_}", [D_MODEL, D_CONV * D_INNER], BF16),
                (f"w_z_{s_}", [D_MODEL, D_INNER], BF16),
                (f"w_xp_{s_}", [D_INNER, 80], BF16),
                (f"w_dtp_{s_}", [DT_RANK, D_INNER], BF16),
                (f"conv_b_{s_}", [D_INNER, 1], F32),
                (f"dt_bias_{s_}", [D_INNER, 1], F32),
                (f"d_skip_{s_}", [D_INNER, 1], F32)):
            dram[nm] = nc.dram_tensor(nm, shape, dt_, kind="ExternalInput").ap()
    dram["w_out"] = nc.dram_tensor("w_out", [D_INNER, D_MODEL], BF16, kind="ExternalInput").ap()
    dram["ident"] = nc.dram_tensor("ident", [128, 128], BF16, kind="ExternalInput").ap()
    out = nc.dram_tensor("out", [D_MODEL, TOK], F32, kind="ExternalOutput").ap()

    ctx = contextlib.ExitStack()
    wpool = ctx.enter_context(tc.tile_pool(name="weights", bufs=1))
    persist = ctx.enter_context(tc.tile_pool(name="persist", bufs=1))
    lnp = ctx.enter_context(tc.tile_pool(name="ln", bufs=4))
    ph1 = ctx.enter_context(tc.tile_pool(name="ph1", bufs=1))    # transient
    ph2 = ctx.enter_context(tc.tile_pool(name="ph2", bufs=2))    # cross-stage
    lat = ctx.enter_context(tc.tile_pool(name="lat", bufs=1))    # big lattice
    latq = ctx.enter_context(tc.tile_pool(name="latq", bufs=2))  # scan inputs
    pp = ctx.enter_context(tc.tile_pool(name="psum", bufs=3, space="PSUM"))
    ppt = ctx.enter_context(tc.tile_pool(name="psumT", bufs=2, space="PSUM"))
    dstage = ctx.enter_context(tc.tile_pool(name="dstage", bufs=2, space="DRAM"))
    dspill = ctx.enter_context(tc.tile_pool(name="dspill", bufs=1, space="DRAM"))

    # ---- load weights into SBUF ----
    wt = {}

    def wload(nm, shape, dt_, src):
        t = wpool.tile(shape, dt_, tag=nm)
        nc.sync.dma_start(t[:], src)
        wt[nm] = t

    for s_ in ("f", "b"):
        wload(f"u_{s_}", [D_MODEL, D_CONV * D_INNER], BF16, dram[f"w_u_{s_}"])
        wload(f"z_{s_}", [D_MODEL, D_INNER], BF16, dram[f"w_z_{s_}"])
        wload(f"dtp_{s_}", [DT_RANK, D_INNER], BF16, dram[f"w_dtp_{s_}"])
        for h in (0, 1):
            hs = slice(h * 96, (h + 1) * 96)
            wload(f"xp_{s_}{h}", [96, 80], BF16, dram[f"w_xp_{s_}"][hs, :])
            wload(f"cb_{s_}{h}", [96, 1], F32, dram[f"conv_b_{s_}"][hs, :])
            wload(f"dtb_{s_}{h}", [96, 1], F32, dram[f"dt_bias_{s_}"][hs, :])
            wload(f"D_{s_}{h}", [96, 1], F32, dram[f"d_skip_{s_}"][hs, :])
    for h in (0, 1):
        wload(f"out{h}", [96, D_MODEL], BF16, dram["w_out"][h * 96:(h + 1) * 96, :])
    wload("ident", [128, 128], BF16, dram["ident"])

    # ---- LN + transpose -> xn_T [96, S, PAD+L] bf16 (3-col zero history
    #      pad per sequence so conv taps never cross seqs). Emitted in
    #      128-column blocks: the first f/b chunks only need blocks 0 and 7,
    #      so the remaining blocks are deferred into the early loop
    #      iterations (their consumers are 2+ iterations downstream). ----
    epst = persist.tile([128, 1], F32, tag="eps")
    nc.vector.memset(epst[:], LN_EPS)
    xn_T = persist.tile([D_MODEL, S * LP], BF16, tag="xnT")
    xn3 = xn_T[:].rearrange("c (s l) -> c s l", s=S)
    nc.vector.memset(xn3[:, :, 0:PAD], 0.0)

    def emit_ln_block(j):
        """LayerNorm the j-th 128-token column block of every sequence."""
        for s in range(S):
            i = s * (Lc // 128) + j
            si, off = s, j * 128
            xt = lnp.tile([128, D_MODEL], F32, tag="ln_x")
            nc.sync.dma_start(xt[:], xtok[i * 128:(i + 1) * 128, :])
            st6 = lnp.tile([128, 6], F32, tag="ln_s6")
            nc.vector.bn_stats(st6[:], xt[:])
            mv = lnp.tile([128, 2], F32, tag="ln_mv")
            nc.vector.bn_aggr(mv[:], st6[:])
            std = lnp.tile([128, 1], F32, tag="ln_sd")
            nc.scalar.activation(std[:], mv[:, 1:2], AF.Sqrt, bias=epst[:])
            rstd = lnp.tile([128, 1], F32, tag="ln_rs")
            nc.vector.reciprocal(rstd[:], std[:])
            xn = lnp.tile([128, D_MODEL], BF16, tag="ln_xn")
            nc.vector.scalar_tensor_tensor(
                xn[:], xt[:], mv[:, 0:1], rstd[:].broadcast_to([128, D_MODEL]),
                OP.subtract, OP.mult)
            pt = ppt.tile([D_MODEL, 128], BF16, tag="tp")
            nc.tensor.transpose(pt[:], xn[:], wt["ident"][:])
            nc.scalar.activation(
                xn3[:, si, PAD + off:PAD + off + 128], pt[:], AF.Copy)

    emit_ln_block(0)
    emit_ln_block(7)

    # ---- persistent small state ----
    carries = {}
    for s_ in ("f", "b"):
        for g in range(2):
            cr = persist.tile([128, D_STATE * D12], BF16, tag=f"carry{s_}{g}")
            nc.vector.memset(cr[:], 0.0)
            carries[(s_, g)] = cr

    yg_dram = {}
    for s_ in ("f", "b"):
        yg_dram[s_] = dspill.tile([D_INNER, S, Lc], BF16, tag=f"ygd{s_}",
                                  name=f"ygdram{s_}")

    veng, geng = nc.vector, nc.gpsimd

    def get_eng(name):
        return {"vector": veng, "gpsimd": geng}[eng[name]]

    def copy_ps(dst3, ps, np_, act=AF.Copy, bias=0.0):
        """One ACT copy: psum [np_, 2, 512] (first SH*T cols each) -> dst [np_, S, T]."""
        nc.scalar.activation(
            dst3.rearrange("p (j s) t -> p j (s t)", j=2),
            ps[0:np_, :, 0:SH * T], act, bias=bias)

    # ---- hoisted lattice-buffer init: zero dA boundary columns (col 0 of
    #      each (n,d12) run) and the pad lanes [96:128] of the g=1 scan
    #      input buffers; neither region is ever rewritten in the loop ----
    dA_t = lat.tile([128, D_STATE, D12, T + 1], BF16, tag="dA")
    nc.vector.memset(dA_t[:, :, :, 0], 0.0)
    for _ in range(2):
        sd = latq.tile([128, 2, D12, T], BF16, tag="sddu", bufs=2)
        nc.vector.memset(sd[96:128], 0.0)
        sb = latq.tile([128, 2, D_STATE, T], BF16, tag="sbc", bufs=2)
        nc.vector.memset(sb[96:128], 0.0)
    # rsrc history pad (cols 0:3 only read at the first backward chunk)
    rsrc0 = ph1.tile([D_MODEL, S, PAD + T], BF16, tag="rsrc")
    nc.vector.memset(rsrc0[:, :, 0:PAD], 0.0)

    # ---------------- software-pipelined main loop ----------------
    # Iteration i's lattice phase (bbuild/scan/pmul/tree on DVE) overlaps
    # iteration i+1's front end (PE matmuls, ACT evacuations, DMA staging).
    # dA exponentials leapfrog: EXP(it,g1) is emitted right after scan(it,g0)
    # frees the single dA buffer, EXP(it+1,g0) right after scan(it,g1).
    iters = [(c, s_) for c in range(NCH) for s_ in ("f", "b")]
    st = {}   # per-iteration tile state

    def emit_front_a(it):
        """rsrc reversal (b only) + u/z tap matmuls + ACT evacuations."""
        c, s_ = it
        v = st.setdefault(it, {})
        if s_ == "f":
            def rhs(j, k):
                return xn3[:, j * SH:(j + 1) * SH, c * T + k:c * T + k + T]
            def rhs_z(j):
                return xn3[:, j * SH:(j + 1) * SH,
                           PAD + c * T:PAD + (c + 1) * T]
        else:
            rsrc = ph1.tile([D_MODEL, S, PAD + T], BF16, tag="rsrc")
            if c == 0:
                nc.vector.tensor_copy(
                    rsrc[:, :, PAD:],
                    xn3[:, :, PAD + Lc - T:PAD + Lc][:, :, ::-1])
            else:
                nc.vector.tensor_copy(
                    rsrc[:],
                    xn3[:, :, PAD + Lc - (c + 1) * T:
                        PAD + Lc - c * T + PAD][:, :, ::-1])
            def rhs(j, k):
                return rsrc[:, j * SH:(j + 1) * SH, k:k + T]
            def rhs_z(j):
                return rsrc[:, j * SH:(j + 1) * SH, PAD:PAD + T]
        v["rsg"] = {}
        for h in (0, 1):
            rs = ph1.tile([96, 2, S, T], BF16, tag=f"rs{h}")
            sg = ph1.tile([96, 2, S, T], BF16, tag=f"sg{h}")
            ps = pp.tile([96, 2, 512], F32, tag="mm")
            for j in range(2):
                for k in range(D_CONV):
                    nc.tensor.matmul(
                        ps[:, j, 0:SH * T],
                        wt[f"u_{s_}"][:, k * D_INNER + h * 96:
                                      k * D_INNER + (h + 1) * 96],
                        rhs(j, k), start=(k == 0), stop=(k == D_CONV - 1))
            copy_ps(sg[:, 0], ps, 96, act=AF.Sigmoid, bias=wt[f"cb_{s_}{h}"][:])
            copy_ps(rs[:, 0], ps, 96, act=AF.Identity, bias=wt[f"cb_{s_}{h}"][:])
            ps = pp.tile([96, 2, 512], F32, tag="mm")
            for j in range(2):
                nc.tensor.matmul(
                    ps[:, j, 0:SH * T],
                    wt[f"z_{s_}"][:, h * 96:(h + 1) * 96],
                    rhs_z(j), start=True, stop=True)
            copy_ps(sg[:, 1], ps, 96, act=AF.Sigmoid)
            copy_ps(rs[:, 1], ps, 96)
            v["rsg"][h] = (rs, sg)

    def emit_front_b1(it):
        """silu mults (DVE) + x_proj + dt_proj matmuls + softplus chain."""
        c, s_ = it
        v = st[it]
        v["ucv"] = {}
        v["szv"] = {}
        for h in (0, 1):
            rs, sg = v["rsg"][h]
            uz = ph2.tile([96, 2, S, T], BF16, tag=f"uz{h}")
            nc.vector.tensor_tensor(uz[:], rs[:], sg[:], OP.mult)
            v["ucv"][h] = uz[:, 0]
            v["szv"][h] = uz[:, 1]
        psx = pp.tile([96, 2, 512], F32, tag="mm")
        for j in range(2):
            for h in (0, 1):
                nc.tensor.matmul(
                    psx[0:80, j, 0:SH * T],
                    wt[f"xp_{s_}{h}"][:],
                    v["ucv"][h][:, j * SH:(j + 1) * SH, :],
                    start=(h == 0), stop=(h == 1))
        dt6 = ph1.tile([DT_RANK, S, T], BF16, tag="dt6")
        copy_ps(dt6[:], psx[0:DT_RANK], DT_RANK)
        bc = ph1.tile([D_STATE, 2, S, T], BF16, tag="bc")
        copy_ps(bc[:, 0], psx[32:32 + D_STATE], D_STATE)
        copy_ps(bc[:, 1], psx[64:64 + D_STATE], D_STATE)
        v["bc"] = bc
        v["ddu"] = {}
        for h in (0, 1):
            psd = pp.tile([96, 2, 512], F32, tag="mm")
            for j in range(2):
                nc.tensor.matmul(
                    psd[:, j, 0:SH * T],
                    wt[f"dtp_{s_}"][:, h * 96:(h + 1) * 96],
                    dt6[:, j * SH:(j + 1) * SH, :],
                    start=True, stop=True)
            pk = ph1.tile([96, 2, S, T], BF16, tag=f"ddu{h}")
            spe = ph1.tile([96, S, T], F32, tag=f"spe{h}")
            copy_ps(spe[:], psd, 96, act=AF.Exp, bias=wt[f"dtb_{s_}{h}"][:])
            nc.scalar.activation(pk[:, 0], spe[:], AF.Ln, bias=1.0)
            v["ddu"][h] = pk

    def emit_front_b2(it):
        """du = delta*uc (DVE) + DRAM staging of the scan-layout shuffle."""
        v = st[it]
        for h in (0, 1):
            pk = v["ddu"][h]
            nc.vector.tensor_tensor(pk[:, 1], pk[:, 0], v["ucv"][h], OP.mult)
        ydu = dstage.tile([2, S, D_INNER, T], BF16, tag="ydu")
        for h in (0, 1):
            for f_ in (0, 1):
                nc.sync.dma_start(
                    ydu[f_, :, h * 96:(h + 1) * 96, :].transpose([1, 0, 2]),
                    v["ddu"][h][:, f_])
        ybc = dstage.tile([2, S, D_STATE, T], BF16, tag="ybc")
        for f_ in (0, 1):
            nc.sync.dma_start(ybc[f_].transpose([1, 0, 2]), v["bc"][:, f_])
        v["ydu"], v["ybc"] = ydu, ybc

    def emit_loads(it, g):
        """sddu/sbc SBUF loads for group g from the DRAM staging."""
        v = st[it]
        sg = SGRP[g]
        soff = 0 if g == 0 else SGRP[0]
        sddu = latq.tile([128, 2, D12, T], BF16, tag="sddu", bufs=2)
        for f_ in (0, 1):
            nc.sync.dma_start(
                sddu[0:16 * sg, f_],
                v["ydu"][f_, soff:soff + sg].rearrange(
                    "s (d16 d12) t -> s d16 d12 t", d16=D16))
        sbc = latq.tile([128, 2, D_STATE, T], BF16, tag="sbc", bufs=2)
        for f_ in (0, 1):
            nc.sync.dma_start(
                sbc[0:16 * sg, f_],
                v["ybc"][f_, soff:soff + sg].unsqueeze(1)
                .broadcast_to([sg, D16, D_STATE, T]))
        v[("sddu", g)], v[("sbc", g)] = sddu, sbc

    def emit_exp(it, g):
        """dA = exp(-a_n*delta); emitted right after the scan freeing dA."""
        v = st[it]
        dA = lat.tile([128, D_STATE, D12, T + 1], BF16, tag="dA")
        for n in range(D_STATE):
            nc.scalar.activation(
                dA[:, n, :, 1:], v[("sddu", g)][:, 0], AF.Exp,
                scale=-float(a_vals[n]))
        v[("dA", g)] = dA

    pnop = persist.tile([1, 1], BF16, tag="pnop")
    tick = persist.tile([1, 1], BF16, tag="tick")

    def emit_bbuild(it, g, gate_src=None):
        """b = du x B on GpSimd; gated (via a tiny Pool copy) to start only
        when the concurrent DVE scan starts, so the Pool SBUF-port traffic
        lands entirely inside the scan window where DVE is insensitive."""
        v = st[it]
        beng = get_eng("bbuild")
        if gate_src is not None and beng is geng:
            nc.vector.memset(tick[:], 0.0)
            nc.gpsimd.tensor_copy(pnop[:], tick[:])
        bt = lat.tile([128, D_STATE, D12, T + 1], BF16, tag="bt", bufs=2)
        beng.tensor_tensor(
            bt[:, :, :, 1:],
            v[("sddu", g)][:, 1].unsqueeze(1).broadcast_to([128, D_STATE, D12, T]),
            v[("sbc", g)][:, 0].unsqueeze(2).broadcast_to([128, D_STATE, D12, T]),
            OP.mult)
        v[("bt", g)] = bt

    def emit_carry_in(it, g):
        c, s_ = it
        nc.vector.tensor_copy(
            st[it][("bt", g)][:, :, :, 0].rearrange("p n d -> p (n d)"),
            carries[(s_, g)][:])

    def emit_scan(it, g):
        """In-place scan: h overwrites b (out aliases data1) — saves a tile."""
        v = st[it]
        bt = v[("bt", g)]
        get_eng("scan").tensor_tensor_scan(
            bt[:].rearrange("p n d t -> p (n d t)"),
            v[("dA", g)][:].rearrange("p n d t -> p (n d t)"),
            bt[:].rearrange("p n d t -> p (n d t)"),
            0.0, OP.mult, OP.add)

    def emit_carry_out(it, g):
        c, s_ = it
        nc.vector.tensor_copy(
            carries[(s_, g)][:],
            st[it][("bt", g)][:, :, :, T].rearrange("p n d -> p (n d)"))

    def emit_reduce(it, g):
        """pmul (shifted in-place: p[t] = h[t+1]*C) + in-place tree + return
        DMAs for group g; everything lives inside bt."""
        v = st[it]
        sg = SGRP[g]
        soff = 0 if g == 0 else SGRP[0]
        bt, sbc = v[("bt", g)], v[("sbc", g)]
        ptl = bt[:, :, :, 0:T]
        get_eng("pmul").tensor_tensor(
            ptl, bt[:, :, :, 1:],
            sbc[:, 1].unsqueeze(2).broadcast_to([128, D_STATE, D12, T]),
            OP.mult)
        teng = get_eng("tree")
        q1 = ptl[:, 8:16]
        teng.tensor_tensor(q1, ptl[:, 0:8], ptl[:, 8:16], OP.add)
        q2 = q1[:, 4:8]
        teng.tensor_tensor(q2, q1[:, 0:4], q1[:, 4:8], OP.add)
        q3 = q2[:, 2:4]
        teng.tensor_tensor(q3, q2[:, 0:2], q2[:, 2:4], OP.add)
        ygt = latq.tile([128, D12, T], BF16, tag="ygt")
        yg_t = ygt[:]
        teng.tensor_tensor(yg_t, q3[:, 0], q3[:, 1], OP.add)
        yy = dstage.tile([8, D_INNER, T], BF16, tag="yy")
        nc.sync.dma_start(yy[0:sg], yg_t[0:16 * sg])
        for h in (0, 1):
            nc.sync.dma_start(
                v["ys_h"][h][:, soff:soff + sg, :],
                yy[0:sg, h * 96:(h + 1) * 96, :].transpose([1, 0, 2]))

    def emit_gating(it):
        c, s_ = it
        v = st[it]
        for h in (0, 1):
            g1 = ph1.tile([96, S, T], BF16, tag=f"g1{h}")
            nc.vector.scalar_tensor_tensor(
                g1[:], v["ucv"][h], wt[f"D_{s_}{h}"][:], v["ys_h"][h][:],
                OP.mult, OP.add)
            yg = ph1.tile([96, S, T], BF16, tag=f"yg{h}")
            nc.vector.tensor_tensor(yg[:], g1[:], v["szv"][h], OP.mult)
            nc.sync.dma_start(
                yg_dram[s_][h * 96:(h + 1) * 96, :, c * T:(c + 1) * T], yg[:])

    # ---- phase 3 (folded into the loop tail): combine dirs, out_proj,
    #      residual for output chunk o — ready once f-chunk o and b-chunk
    #      NCH-1-o have both been written to yg_dram ----
    x_T3 = x_T.rearrange("c (s l) -> c s l", s=S)
    out3 = out.rearrange("c (s l) -> c s l", s=S)

    def emit_phase3(o):
        yt = {}
        for h in (0, 1):
            ygf = ph1.tile([96, S, T], BF16, tag=f"p3f{h}")
            nc.sync.dma_start(ygf[:], yg_dram["f"][h * 96:(h + 1) * 96, :, o * T:(o + 1) * T])
            ygb = ph1.tile([96, S, T], BF16, tag=f"yg{h}")
            nc.sync.dma_start(ygb[:], yg_dram["b"][h * 96:(h + 1) * 96, :, Lc - (o + 1) * T:Lc - o * T])
            ysum = ph1.tile([96, S, T], BF16, tag=f"g1{h}")
            nc.vector.tensor_tensor(ysum[:], ygf[:], ygb[:, :, ::-1], OP.add)
            yt[h] = ysum
        pso = pp.tile([96, 2, 512], F32, tag="mm")
        for j in range(2):
            for h in (0, 1):
                nc.tensor.matmul(
                    pso[:, j, 0:SH * T], wt[f"out{h}"][:],
                    yt[h][:, j * SH:(j + 1) * SH, :], start=(h == 0), stop=(h == 1))
        xc = ph1.tile([96, S, T], F32, tag="spe0")
        nc.sync.dma_start(xc[:], x_T3[:, :, o * T:(o + 1) * T])
        oc = ph1.tile([96, S, T], F32, tag="spe1")
        nc.vector.tensor_tensor(
            oc[:].rearrange("p (j s) t -> p j (s t)", j=2),
            pso[:, :, 0:SH * T],
            xc[:].rearrange("p (j s) t -> p j (s t)", j=2), OP.add)
        nc.sync.dma_start(out3[:, :, o * T:(o + 1) * T], oc[:])

    # pipeline prologue: fully stage iteration 0, front_a of iteration 1
    emit_front_a(iters[0])
    emit_front_b1(iters[0])
    emit_front_b2(iters[0])
    emit_loads(iters[0], 0)
    emit_loads(iters[0], 1)
    emit_exp(iters[0], 0)
    emit_bbuild(iters[0], 0)
    emit_front_a(iters[1])
    for idx, it in enumerate(iters):
        nxt = iters[idx + 1] if idx + 1 < len(iters) else None
        nxt2 = iters[idx + 2] if idx + 2 < len(iters) else None
        v = st[it]
        v["ys_h"] = {h: ph2.tile([96, S, T], BF16, tag=f"ysh{h}",
                                 name=f"ysh{h}") for h in (0, 1)}
        # ---- group 0 ----
        emit_carry_in(it, 0)
        emit_bbuild(it, 1, gate_src=True)
        emit_scan(it, 0)            # Pool bbuild(it,1) runs under this scan
        emit_exp(it, 1)
        emit_carry_out(it, 0)
        if nxt is not None:
            emit_front_b1(nxt)      # silu mults + xp/dt chain (front_a ran
                                    # during the previous iteration's g1)
        emit_reduce(it, 0)
        if idx in (0, 2, 4):
            emit_ln_block(idx // 2 + 1)     # deferred LN: consumers are
            emit_ln_block(6 - idx // 2)     # >= 2 iterations downstream
        if nxt is not None:
            emit_front_b2(nxt)      # du mult + DRAM staging
            emit_loads(nxt, 0)
        # ---- group 1 ----
        emit_carry_in(it, 1)
        if nxt2 is not None:
            emit_front_a(nxt2)      # PE/ACT front two iterations ahead
        emit_scan(it, 1)
        if nxt is not None:
            emit_exp(nxt, 0)
        emit_carry_out(it, 1)
        emit_reduce(it, 1)
        if nxt is not None:
            emit_bbuild(nxt, 0)     # loads(nxt,0) have long landed by now
            emit_loads(nxt, 1)
        emit_gating(it)
        if it[1] == "b" and it[0] >= NCH // 2:
            emit_phase3(it[0])
            emit_phase3(NCH - 1 - it[0])
        prev = iters[idx - 1] if idx else None
        if prev in st:
            del st[prev]

    ctx.close()


# ---------------- host side ----------------

def _prep_params(inputs):
    bf = ml_dtypes.bfloat16
    p = {}
    ln_w = inputs["ln_w"].astype(np.float64)
    assert np.abs(inputs["ln_b"]).max() == 0.0, "ln_b folding not implemented"
    for s_ in ("f", "b"):
        w = inputs[f"in_proj_w_{s_}"].astype(np.float64) * ln_w[None, :]
        wu = w[0:D_INNER]                       # [192, 96]
        cw = inputs[f"conv_w_{s_}"].astype(np.float64)  # [192, 4]
        taps = [np.ascontiguousarray(wu.T) * cw[:, k][None, :]
                for k in range(D_CONV)]         # each [96, 192]
        p[f"w_u_{s_}"] = np.concatenate(taps, axis=1).astype(bf)
        p[f"w_z_{s_}"] = np.ascontiguousarray(w[D_INNER:].T).astype(bf)
        xp = np.zeros((D_INNER, 80), np.float32)
        xpw = inputs[f"x_proj_w_{s_}"]          # [38, 192]
        xp[:, 0:DT_RANK] = xpw[0:DT_RANK].T
        xp[:, 32:32 + D_STATE] = xpw[DT_RANK:DT_RANK + D_STATE].T
        xp[:, 64:64 + D_STATE] = xpw[DT_RANK + D_STATE:].T
        p[f"w_xp_{s_}"] = xp.astype(bf)
        p[f"w_dtp_{s_}"] = np.ascontiguousarray(inputs[f"dt_proj_w_{s_}"].T).astype(bf)
        p[f"conv_b_{s_}"] = inputs[f"conv_b_{s_}"].reshape(D_INNER, 1).astype(np.float32)
        p[f"dt_bias_{s_}"] = inputs[f"dt_bias_{s_}"].reshape(D_INNER, 1).astype(np.float32)
        p[f"d_skip_{s_}"] = inputs[f"D_{s_}"].reshape(D_INNER, 1).astype(np.float32)
    p["w_out"] = np.ascontiguousarray(inputs["out_proj_w"].T).astype(bf)
    p["ident"] = np.eye(128, dtype=bf)
    a_f = np.exp(inputs["A_log_f"][0]).astype(np.float32)
    assert np.allclose(np.exp(inputs["A_log_f"]), np.tile(a_f, (D_INNER, 1)))
    assert np.allclose(np.exp(inputs["A_log_b"]), np.tile(a_f, (D_INNER, 1)))
    p["_a_vals"] = [float(v) for v in a_f]
    return p


def _pixel_shuffle(x):
    B, C, H, W = x.shape
    nh, nw = H // P_PIX, W // P_PIX
    xd = x.reshape(B, C, nh, P_PIX, nw, P_PIX).transpose(0, 3, 5, 1, 2, 4)
    return xd.reshape(B * P_PIX * P_PIX, C, nh * nw)


def _pixel_unshuffle(y):
    nh = nw = NH
    x = y.reshape(1, P_PIX, P_PIX, D_MODEL, nh, nw).transpose(0, 3, 4, 1, 5, 2)
    return np.ascontiguousarray(x.reshape(1, D_MODEL, HW_, HW_))


_COMPILED = {}


def _split_dma_waits(nc, max_waits=1):
    """The HW pseudo-DMA supports at most 2 sem waits; move the rest onto a
    preceding NoOp on the issuing engine (same semantics, program order)."""
    nid = [0]
    for f in nc.m.functions:
        for b in f.blocks:
            il = b.instructions
            out = []
            changed = False
            for inst in il:
                si = getattr(inst, "sync_info", None)
                if (type(inst).__name__ != "InstNoOp" and si is not None
                        and si.on_wait is not None and len(si.on_wait) > max_waits):
                    excess = list(si.on_wait[:-max_waits])
                    keep = list(si.on_wait[-max_waits:])
                    for w in excess:
                        nop = mybir.InstNoOp(
                            name=f"dmawait-nop-{nid[0]}", engine=inst.engine,
                            ins=[], outs=[],
                            sync_info=mybir.SyncInfo(on_wait=[w], on_update=[]))
                        nid[0] += 1
                        out.append(nop)
                    inst.sync_info = mybir.SyncInfo(
                        on_wait=keep, on_update=list(si.on_update or []))
                    changed = True
                out.append(inst)
            if changed:
                b.instructions = out


def _get_compiled(cfg, a_vals, engines=None, split_waits=True):
    key = (cfg.L, cfg.T, cfg.S, tuple(a_vals), str(engines), split_waits)
    if key not in _COMPILED:
        nc = bass.Bass("TRN2", target_bir_lowering=False, debug=False)
        with tile.TileContext(nc) as tc:
            build_kernel(nc, tc, cfg, a_vals, engines=engines)
        if split_waits:
            _split_dma_waits(nc)
        _COMPILED[key] = nc
    return _COMPILED[key]


COUNTS = [13, 13, 13, 13, 12, 12, 12, 12]


def make_in_maps(x, p, cfg):
    xs = _pixel_shuffle(x.astype(np.float32))
    in_maps = []
    off = 0
    S = cfg.S
    for ci in range(NCORES):
        cnt = COUNTS[ci]
        sl = xs[off:off + cnt]
        off += cnt
        if cnt < S:
            sl = np.concatenate([sl, np.zeros((S - cnt, D_MODEL, cfg.L), np.float32)], 0)
        m = {"xtok": np.ascontiguousarray(sl.transpose(0, 2, 1).reshape(cfg.TOK, D_MODEL)),
             "x_T": np.ascontiguousarray(sl.transpose(1, 0, 2).reshape(D_MODEL, cfg.TOK))}
        m.update(p)
        in_maps.append(m)
    return in_maps


def kernel(**inputs):
    inputs = {k: np.asarray(v) for k, v in inputs.items()}
    x = inputs["x"]
    cfg = Cfg()
    p = _prep_params(inputs)
    a_vals = p.pop("_a_vals")
    in_maps = make_in_maps(x, p, cfg)
    nc = _get_compiled(cfg, a_vals)
    res = run_bass_kernel_spmd(nc, in_maps, list(range(NCORES)))
    y = np.empty((NB, D_MODEL, L_FULL), np.float32)
    off = 0
    for ci in range(NCORES):
        o = np.asarray(res.results[ci]["out"]).reshape(D_MODEL, cfg.S, L_FULL)
        cnt = COUNTS[ci]
        y[off:off + cnt] = o.transpose(1, 0, 2)[:cnt]
        off += cnt
    return _pixel_unshuffle(y).astype(x.dtype)


# revision 33
# speedup vs baseline: 1.0036x; 1.0036x over previous
"""Trainium2 Bass kernel for nn_BiPixelMambaLayer.

Self-contained: takes the FULL unsharded inputs (as produced by the problem's
setup_inputs), shards the NB=100 pixel-shuffled sequences across 8 NeuronCores,
runs a Bass/Tile kernel per core, and reassembles the full output.

Per-core algorithm (S=14 sequence slots of length L=1024, d_model=96):
  LN -> fused in_proj+causal depthwise conv (4 shifted PE matmuls with
  tap-folded weights, PSUM accumulate) -> Silu (direct from PSUM)
  -> x_proj -> dt_proj -> Softplus (direct from PSUM)
  -> exact selective scan (chunked, carry columns, bf16 lattice, DVE
     tensor_tensor_scan over flattened (n, d12, t) runs with zero-dA
     boundary columns) -> C-contraction (tree reduce over n) -> gating
  -> out_proj -> +residual.

Scan layout: partition p = s*16 + d16 (8 seqs x 16), free = (n=16, d12=12, t),
with d = d16*12 + d12.  A(d, n) = -exp(A_log)[0, n] is constant across d
(S4D init); the exact per-n fp32 decay rates are baked in as ACT Exp scales.

Engine budget: the DVE scan (2.1 ns/elem, serial recurrence) is the hard
floor; everything else is pushed to PE (conv taps), ACT (exp/silu/softplus,
PSUM evacuation), GpSimd (small carry copies) or hoisted out of the loop
(pad/boundary memsets).
"""
import contextlib
import numpy as np
import ml_dtypes

import concourse.bass as bass
import concourse.tile as tile
from concourse import mybir
from concourse.bass_utils import run_bass_kernel_spmd

BF16 = mybir.dt.bfloat16
F32 = mybir.dt.float32
AF = mybir.ActivationFunctionType
OP = mybir.AluOpType

# ---------------- problem constants ----------------
D_MODEL = 96
D_STATE = 16      # n
D_CONV = 4
D_INNER = 192     # d
DT_RANK = 6
P_PIX = 10
LN_EPS = 1e-5
HW_ = 320
NH = HW_ // P_PIX           # 32
L_FULL = NH * NH            # 1024
NB = 100
NCORES = 8
D16 = 16
D12 = 12
SGRP = (8, 6)               # sequence groups over S=14 (partitions = s*16+d16)
PAD = D_CONV - 1            # 3 history columns per sequence


class Cfg:
    def __init__(self, L=L_FULL, T=64, S=14):
        assert L % T == 0
        self.L = L
        self.T = T
        self.NCH = L // T
        self.S = S
        self.TOK = S * L
        self.SH = S // 2            # 7 per split
        self.LP = L + PAD           # padded per-sequence stride in xn_T


# ---------------- device kernel ----------------

def build_kernel(nc, tc, cfg, a_vals, engines=None):
    """Emit the full per-core kernel into nc (inside TileContext tc).

    a_vals: 16 positive floats = exp(A_log)[0, :] (decay rate per state n).
    """
    eng = {"bbuild": "vector", "pmul": "vector", "tree": "vector",
           "scan": "vector"}
    if engines:
        eng.update(engines)
    T, NCH, S, TOK, Lc, SH, LP = (cfg.T, cfg.NCH, cfg.S, cfg.TOK, cfg.L,
                                  cfg.SH, cfg.LP)

    # ---- DRAM I/O ----
    xtok = nc.dram_tensor("xtok", [TOK, D_MODEL], F32, kind="ExternalInput").ap()
    x_T = nc.dram_tensor("x_T", [D_MODEL, TOK], F32, kind="ExternalInput").ap()
    dram = {}
    for s_ in ("f", "b"):
        for nm, shape, dt_ in (
                (f"w_u_{s_}", [D_MODEL, D_CONV * D_INNER], BF16),
                (f"w_z_{s_}", [D_MODEL, D_INNER], BF16),
                (f"w_xp_{s_}", [D_INNER, 80], BF16),
                (f"w_dtp_{s_}", [DT_RANK, D_INNER], BF16),
                (f"conv_b_{s_}", [D_INNER, 1], F32),
                (f"dt_bias_{s_}", [D_INNER, 1], F32),
                (f"d_skip_{s_}", [D_INNER, 1], F32)):
            dram[nm] = nc.dram_tensor(nm, shape, dt_, kind="ExternalInput").ap()
    dram["w_out"] = nc.dram_tensor("w_out", [D_INNER, D_MODEL], BF16, kind="ExternalInput").ap()
    dram["ident"] = nc.dram_tensor("ident", [128, 128], BF16, kind="ExternalInput").ap()
    out = nc.dram_tensor("out", [D_MODEL, TOK], F32, kind="ExternalOutput").ap()

    ctx = contextlib.ExitStack()
    wpool = ctx.enter_context(tc.tile_pool(name="weights", bufs=1))
    persist = ctx.enter_context(tc.tile_pool(name="persist", bufs=1))
    lnp = ctx.enter_context(tc.tile_pool(name="ln", bufs=4))
    ph1 = ctx.enter_context(tc.tile_pool(name="ph1", bufs=1))    # transient
    ph2 = ctx.enter_context(tc.tile_pool(name="ph2", bufs=2))    # cross-stage
    lat = ctx.enter_context(tc.tile_pool(name="lat", bufs=1))    # big lattice
    latq = ctx.enter_context(tc.tile_pool(name="latq", bufs=2))  # scan inputs
    pp = ctx.enter_context(tc.tile_pool(name="psum", bufs=3, space="PSUM"))
    ppt = ctx.enter_context(tc.tile_pool(name="psumT", bufs=2, space="PSUM"))
    dstage = ctx.enter_context(tc.tile_pool(name="dstage", bufs=2, space="DRAM"))
    dspill = ctx.enter_context(tc.tile_pool(name="dspill", bufs=1, space="DRAM"))

    # ---- load weights into SBUF ----
    wt = {}

    def wload(nm, shape, dt_, src):
        t = wpool.tile(shape, dt_, tag=nm)
        nc.sync.dma_start(t[:], src)
        wt[nm] = t

    for s_ in ("f", "b"):
        wload(f"u_{s_}", [D_MODEL, D_CONV * D_INNER], BF16, dram[f"w_u_{s_}"])
        wload(f"z_{s_}", [D_MODEL, D_INNER], BF16, dram[f"w_z_{s_}"])
        wload(f"dtp_{s_}", [DT_RANK, D_INNER], BF16, dram[f"w_dtp_{s_}"])
        for h in (0, 1):
            hs = slice(h * 96, (h + 1) * 96)
            wload(f"xp_{s_}{h}", [96, 80], BF16, dram[f"w_xp_{s_}"][hs, :])
            wload(f"cb_{s_}{h}", [96, 1], F32, dram[f"conv_b_{s_}"][hs, :])
            wload(f"dtb_{s_}{h}", [96, 1], F32, dram[f"dt_bias_{s_}"][hs, :])
            wload(f"D_{s_}{h}", [96, 1], F32, dram[f"d_skip_{s_}"][hs, :])
    for h in (0, 1):
        wload(f"out{h}", [96, D_MODEL], BF16, dram["w_out"][h * 96:(h + 1) * 96, :])
    wload("ident", [128, 128], BF16, dram["ident"])

    # ---- LN + transpose -> xn_T [96, S, PAD+L] bf16 (3-col zero history
    #      pad per sequence so conv taps never cross seqs). Emitted in
    #      128-column blocks: the first f/b chunks only need blocks 0 and 7,
    #      so the remaining blocks are deferred into the early loop
    #      iterations (their consumers are 2+ iterations downstream). ----
    epst = persist.tile([128, 1], F32, tag="eps")
    nc.vector.memset(epst[:], LN_EPS)
    xn_T = persist.tile([D_MODEL, S * LP], BF16, tag="xnT")
    xn3 = xn_T[:].rearrange("c (s l) -> c s l", s=S)
    nc.vector.memset(xn3[:, :, 0:PAD], 0.0)

    def emit_ln_block(j):
        """LayerNorm the j-th 128-token column block of every sequence."""
        for s in range(S):
            i = s * (Lc // 128) + j
            si, off = s, j * 128
            xt = lnp.tile([128, D_MODEL], F32, tag="ln_x")
            nc.sync.dma_start(xt[:], xtok[i * 128:(i + 1) * 128, :])
            st6 = lnp.tile([128, 6], F32, tag="ln_s6")
            nc.vector.bn_stats(st6[:], xt[:])
            mv = lnp.tile([128, 2], F32, tag="ln_mv")
            nc.vector.bn_aggr(mv[:], st6[:])
            std = lnp.tile([128, 1], F32, tag="ln_sd")
            nc.scalar.activation(std[:], mv[:, 1:2], AF.Sqrt, bias=epst[:])
            rstd = lnp.tile([128, 1], F32, tag="ln_rs")
            nc.vector.reciprocal(rstd[:], std[:])
            xn = lnp.tile([128, D_MODEL], BF16, tag="ln_xn")
            nc.vector.scalar_tensor_tensor(
                xn[:], xt[:], mv[:, 0:1], rstd[:].broadcast_to([128, D_MODEL]),
                OP.subtract, OP.mult)
            pt = ppt.tile([D_MODEL, 128], BF16, tag="tp")
            nc.tensor.transpose(pt[:], xn[:], wt["ident"][:])
            nc.scalar.activation(
                xn3[:, si, PAD + off:PAD + off + 128], pt[:], AF.Copy)

    emit_ln_block(0)
    emit_ln_block(7)

    # ---- persistent small state ----
    carries = {}
    for s_ in ("f", "b"):
        for g in range(2):
            cr = persist.tile([128, D_STATE * D12], BF16, tag=f"carry{s_}{g}")
            nc.vector.memset(cr[:], 0.0)
            carries[(s_, g)] = cr

    yg_dram = {}
    for s_ in ("f", "b"):
        yg_dram[s_] = dspill.tile([D_INNER, S, Lc], BF16, tag=f"ygd{s_}",
                                  name=f"ygdram{s_}")

    veng, geng = nc.vector, nc.gpsimd

    def get_eng(name):
        return {"vector": veng, "gpsimd": geng}[eng[name]]

    def copy_ps(dst3, ps, np_, act=AF.Copy, bias=0.0):
        """One ACT copy: psum [np_, 2, 512] (first SH*T cols each) -> dst [np_, S, T]."""
        nc.scalar.activation(
            dst3.rearrange("p (j s) t -> p j (s t)", j=2),
            ps[0:np_, :, 0:SH * T], act, bias=bias)

    # ---- hoisted lattice-buffer init: zero dA boundary columns (col 0 of
    #      each (n,d12) run) and the pad lanes [96:128] of the g=1 scan
    #      input buffers; neither region is ever rewritten in the loop ----
    dA_t = lat.tile([128, D_STATE, D12, T + 1], BF16, tag="dA")
    nc.vector.memset(dA_t[:, :, :, 0], 0.0)
    for _ in range(2):
        sd = latq.tile([128, 2, D12, T], BF16, tag="sddu", bufs=2)
        nc.vector.memset(sd[96:128], 0.0)
        sb = latq.tile([128, 2, D_STATE, T], BF16, tag="sbc", bufs=2)
        nc.vector.memset(sb[96:128], 0.0)
    # rsrc history pad (cols 0:3 only read at the first backward chunk)
    rsrc0 = ph1.tile([D_MODEL, S, PAD + T], BF16, tag="rsrc")
    nc.vector.memset(rsrc0[:, :, 0:PAD], 0.0)

    # ---------------- software-pipelined main loop ----------------
    # Iteration i's lattice phase (bbuild/scan/pmul/tree on DVE) overlaps
    # iteration i+1's front end (PE matmuls, ACT evacuations, DMA staging).
    # dA exponentials leapfrog: EXP(it,g1) is emitted right after scan(it,g0)
    # frees the single dA buffer, EXP(it+1,g0) right after scan(it,g1).
    iters = [(c, s_) for c in range(NCH) for s_ in ("f", "b")]
    st = {}   # per-iteration tile state

    def emit_front_a(it):
        """rsrc reversal (b only) + u/z tap matmuls + ACT evacuations."""
        c, s_ = it
        v = st.setdefault(it, {})
        if s_ == "f":
            def rhs(j, k):
                return xn3[:, j * SH:(j + 1) * SH, c * T + k:c * T + k + T]
            def rhs_z(j):
                return xn3[:, j * SH:(j + 1) * SH,
                           PAD + c * T:PAD + (c + 1) * T]
        else:
            rsrc = ph1.tile([D_MODEL, S, PAD + T], BF16, tag="rsrc")
            if c == 0:
                nc.vector.tensor_copy(
                    rsrc[:, :, PAD:],
                    xn3[:, :, PAD + Lc - T:PAD + Lc][:, :, ::-1])
            else:
                nc.vector.tensor_copy(
                    rsrc[:],
                    xn3[:, :, PAD + Lc - (c + 1) * T:
                        PAD + Lc - c * T + PAD][:, :, ::-1])
            def rhs(j, k):
                return rsrc[:, j * SH:(j + 1) * SH, k:k + T]
            def rhs_z(j):
                return rsrc[:, j * SH:(j + 1) * SH, PAD:PAD + T]
        v["ucsr"] = {}
        for h in (0, 1):
            ps = pp.tile([96, 2, 512], F32, tag="mm")
            for j in range(2):
                for k in range(D_CONV):
                    nc.tensor.matmul(
                        ps[:, j, 0:SH * T],
                        wt[f"u_{s_}"][:, k * D_INNER + h * 96:
                                      k * D_INNER + (h + 1) * 96],
                        rhs(j, k), start=(k == 0), stop=(k == D_CONV - 1))
            ucs = ph1.tile([96, S, T], BF16, tag=f"ucs{h}")
            copy_ps(ucs[:], ps, 96, act=AF.Sigmoid, bias=wt[f"cb_{s_}{h}"][:])
            ucr = ph1.tile([96, S, T], BF16, tag=f"ucr{h}")
            copy_ps(ucr[:], ps, 96, act=AF.Identity, bias=wt[f"cb_{s_}{h}"][:])
            v["ucsr"][h] = (ucs, ucr)
        v["szsr"] = {}
        for h in (0, 1):
            ps = pp.tile([96, 2, 512], F32, tag="mm")
            for j in range(2):
                nc.tensor.matmul(
                    ps[:, j, 0:SH * T],
                    wt[f"z_{s_}"][:, h * 96:(h + 1) * 96],
                    rhs_z(j), start=True, stop=True)
            szs = ph1.tile([96, S, T], BF16, tag=f"szs{h}")
            copy_ps(szs[:], ps, 96, act=AF.Sigmoid)
            szr = ph1.tile([96, S, T], BF16, tag=f"szr{h}")
            copy_ps(szr[:], ps, 96)
            v["szsr"][h] = (szs, szr)

    def emit_front_b1(it):
        """silu mults (DVE) + x_proj + dt_proj matmuls + softplus chain."""
        c, s_ = it
        v = st[it]
        v["ucv"] = {}
        v["szv"] = {}
        for h in (0, 1):
            ucs, ucr = v["ucsr"][h]
            uc = ph2.tile([96, S, T], BF16, tag=f"uc{h}")
            nc.vector.tensor_tensor(uc[:], ucr[:], ucs[:], OP.mult)
            v["ucv"][h] = uc
            szs, szr = v["szsr"][h]
            sz = ph2.tile([96, S, T], BF16, tag=f"sz{h}")
            nc.vector.tensor_tensor(sz[:], szr[:], szs[:], OP.mult)
            v["szv"][h] = sz
        psx = pp.tile([96, 2, 512], F32, tag="mm")
        for j in range(2):
            for h in (0, 1):
                nc.tensor.matmul(
                    psx[0:80, j, 0:SH * T],
                    wt[f"xp_{s_}{h}"][:],
                    v["ucv"][h][:, j * SH:(j + 1) * SH, :],
                    start=(h == 0), stop=(h == 1))
        dt6 = ph1.tile([DT_RANK, S, T], BF16, tag="dt6")
        copy_ps(dt6[:], psx[0:DT_RANK], DT_RANK)
        bc = ph1.tile([D_STATE, 2, S, T], BF16, tag="bc")
        copy_ps(bc[:, 0], psx[32:32 + D_STATE], D_STATE)
        copy_ps(bc[:, 1], psx[64:64 + D_STATE], D_STATE)
        v["bc"] = bc
        v["ddu"] = {}
        for h in (0, 1):
            psd = pp.tile([96, 2, 512], F32, tag="mm")
            for j in range(2):
                nc.tensor.matmul(
                    psd[:, j, 0:SH * T],
                    wt[f"dtp_{s_}"][:, h * 96:(h + 1) * 96],
                    dt6[:, j * SH:(j + 1) * SH, :],
                    start=True, stop=True)
            pk = ph1.tile([96, 2, S, T], BF16, tag=f"ddu{h}")
            spe = ph1.tile([96, S, T], F32, tag=f"spe{h}")
            copy_ps(spe[:], psd, 96, act=AF.Exp, bias=wt[f"dtb_{s_}{h}"][:])
            nc.scalar.activation(pk[:, 0], spe[:], AF.Ln, bias=1.0)
            v["ddu"][h] = pk

    def emit_front_b2(it):
        """du = delta*uc (DVE) + DRAM staging of the scan-layout shuffle."""
        v = st[it]
        for h in (0, 1):
            pk = v["ddu"][h]
            nc.vector.tensor_tensor(pk[:, 1], pk[:, 0], v["ucv"][h][:], OP.mult)
        ydu = dstage.tile([2, S, D_INNER, T], BF16, tag="ydu")
        for h in (0, 1):
            for f_ in (0, 1):
                nc.sync.dma_start(
                    ydu[f_, :, h * 96:(h + 1) * 96, :].transpose([1, 0, 2]),
                    v["ddu"][h][:, f_])
        ybc = dstage.tile([2, S, D_STATE, T], BF16, tag="ybc")
        for f_ in (0, 1):
            nc.sync.dma_start(ybc[f_].transpose([1, 0, 2]), v["bc"][:, f_])
        v["ydu"], v["ybc"] = ydu, ybc

    def emit_loads(it, g):
        """sddu/sbc SBUF loads for group g from the DRAM staging."""
        v = st[it]
        sg = SGRP[g]
        soff = 0 if g == 0 else SGRP[0]
        sddu = latq.tile([128, 2, D12, T], BF16, tag="sddu", bufs=2)
        for f_ in (0, 1):
            nc.sync.dma_start(
                sddu[0:16 * sg, f_],
                v["ydu"][f_, soff:soff + sg].rearrange(
                    "s (d16 d12) t -> s d16 d12 t", d16=D16))
        sbc = latq.tile([128, 2, D_STATE, T], BF16, tag="sbc", bufs=2)
        for f_ in (0, 1):
            nc.sync.dma_start(
                sbc[0:16 * sg, f_],
                v["ybc"][f_, soff:soff + sg].unsqueeze(1)
                .broadcast_to([sg, D16, D_STATE, T]))
        v[("sddu", g)], v[("sbc", g)] = sddu, sbc

    def emit_exp(it, g):
        """dA = exp(-a_n*delta); emitted right after the scan freeing dA."""
        v = st[it]
        dA = lat.tile([128, D_STATE, D12, T + 1], BF16, tag="dA")
        for n in range(D_STATE):
            nc.scalar.activation(
                dA[:, n, :, 1:], v[("sddu", g)][:, 0], AF.Exp,
                scale=-float(a_vals[n]))
        v[("dA", g)] = dA

    pnop = persist.tile([1, 1], BF16, tag="pnop")
    tick = persist.tile([1, 1], BF16, tag="tick")

    def emit_bbuild(it, g, gate_src=None):
        """b = du x B on GpSimd; gated (via a tiny Pool copy) to start only
        when the concurrent DVE scan starts, so the Pool SBUF-port traffic
        lands entirely inside the scan window where DVE is insensitive."""
        v = st[it]
        beng = get_eng("bbuild")
        if gate_src is not None and beng is geng:
            nc.vector.memset(tick[:], 0.0)
            nc.gpsimd.tensor_copy(pnop[:], tick[:])
        bt = lat.tile([128, D_STATE, D12, T + 1], BF16, tag="bt", bufs=2)
        beng.tensor_tensor(
            bt[:, :, :, 1:],
            v[("sddu", g)][:, 1].unsqueeze(1).broadcast_to([128, D_STATE, D12, T]),
            v[("sbc", g)][:, 0].unsqueeze(2).broadcast_to([128, D_STATE, D12, T]),
            OP.mult)
        v[("bt", g)] = bt

    def emit_carry_in(it, g):
        c, s_ = it
        nc.vector.tensor_copy(
            st[it][("bt", g)][:, :, :, 0].rearrange("p n d -> p (n d)"),
            carries[(s_, g)][:])

    def emit_scan(it, g):
        """In-place scan: h overwrites b (out aliases data1) — saves a tile."""
        v = st[it]
        bt = v[("bt", g)]
        get_eng("scan").tensor_tensor_scan(
            bt[:].rearrange("p n d t -> p (n d t)"),
            v[("dA", g)][:].rearrange("p n d t -> p (n d t)"),
            bt[:].rearrange("p n d t -> p (n d t)"),
            0.0, OP.mult, OP.add)

    def emit_carry_out(it, g):
        c, s_ = it
        nc.vector.tensor_copy(
            carries[(s_, g)][:],
            st[it][("bt", g)][:, :, :, T].rearrange("p n d -> p (n d)"))

    def emit_reduce(it, g):
        """pmul (shifted in-place: p[t] = h[t+1]*C) + in-place tree + return
        DMAs for group g; everything lives inside bt."""
        v = st[it]
        sg = SGRP[g]
        soff = 0 if g == 0 else SGRP[0]
        bt, sbc = v[("bt", g)], v[("sbc", g)]
        ptl = bt[:, :, :, 0:T]
        get_eng("pmul").tensor_tensor(
            ptl, bt[:, :, :, 1:],
            sbc[:, 1].unsqueeze(2).broadcast_to([128, D_STATE, D12, T]),
            OP.mult)
        teng = get_eng("tree")
        q1 = ptl[:, 8:16]
        teng.tensor_tensor(q1, ptl[:, 0:8], ptl[:, 8:16], OP.add)
        q2 = q1[:, 4:8]
        teng.tensor_tensor(q2, q1[:, 0:4], q1[:, 4:8], OP.add)
        q3 = q2[:, 2:4]
        teng.tensor_tensor(q3, q2[:, 0:2], q2[:, 2:4], OP.add)
        ygt = latq.tile([128, D12, T], BF16, tag="ygt")
        yg_t = ygt[:]
        teng.tensor_tensor(yg_t, q3[:, 0], q3[:, 1], OP.add)
        yy = dstage.tile([8, D_INNER, T], BF16, tag="yy")
        nc.sync.dma_start(yy[0:sg], yg_t[0:16 * sg])
        for h in (0, 1):
            nc.sync.dma_start(
                v["ys_h"][h][:, soff:soff + sg, :],
                yy[0:sg, h * 96:(h + 1) * 96, :].transpose([1, 0, 2]))

    def emit_gating(it):
        c, s_ = it
        v = st[it]
        for h in (0, 1):
            g1 = ph1.tile([96, S, T], BF16, tag=f"g1{h}")
            nc.vector.scalar_tensor_tensor(
                g1[:], v["ucv"][h][:], wt[f"D_{s_}{h}"][:], v["ys_h"][h][:],
                OP.mult, OP.add)
            yg = ph1.tile([96, S, T], BF16, tag=f"yg{h}")
            nc.vector.tensor_tensor(yg[:], g1[:], v["szv"][h][:], OP.mult)
            nc.sync.dma_start(
                yg_dram[s_][h * 96:(h + 1) * 96, :, c * T:(c + 1) * T], yg[:])

    # ---- phase 3 (folded into the loop tail): combine dirs, out_proj,
    #      residual for output chunk o — ready once f-chunk o and b-chunk
    #      NCH-1-o have both been written to yg_dram ----
    x_T3 = x_T.rearrange("c (s l) -> c s l", s=S)
    out3 = out.rearrange("c (s l) -> c s l", s=S)

    def emit_phase3(o):
        yt = {}
        for h in (0, 1):
            ygf = ph1.tile([96, S, T], BF16, tag=f"p3f{h}")
            nc.sync.dma_start(ygf[:], yg_dram["f"][h * 96:(h + 1) * 96, :, o * T:(o + 1) * T])
            ygb = ph1.tile([96, S, T], BF16, tag=f"yg{h}")
            nc.sync.dma_start(ygb[:], yg_dram["b"][h * 96:(h + 1) * 96, :, Lc - (o + 1) * T:Lc - o * T])
            ysum = ph1.tile([96, S, T], BF16, tag=f"g1{h}")
            nc.vector.tensor_tensor(ysum[:], ygf[:], ygb[:, :, ::-1], OP.add)
            yt[h] = ysum
        pso = pp.tile([96, 2, 512], F32, tag="mm")
        for j in range(2):
            for h in (0, 1):
                nc.tensor.matmul(
                    pso[:, j, 0:SH * T], wt[f"out{h}"][:],
                    yt[h][:, j * SH:(j + 1) * SH, :], start=(h == 0), stop=(h == 1))
        xc = ph1.tile([96, S, T], F32, tag="spe0")
        nc.sync.dma_start(xc[:], x_T3[:, :, o * T:(o + 1) * T])
        oc = ph1.tile([96, S, T], F32, tag="spe1")
        nc.vector.tensor_tensor(
            oc[:].rearrange("p (j s) t -> p j (s t)", j=2),
            pso[:, :, 0:SH * T],
            xc[:].rearrange("p (j s) t -> p j (s t)", j=2), OP.add)
        nc.sync.dma_start(out3[:, :, o * T:(o + 1) * T], oc[:])

    # pipeline prologue: fully stage iteration 0, front_a of iteration 1
    emit_front_a(iters[0])
    emit_front_b1(iters[0])
    emit_front_b2(iters[0])
    emit_loads(iters[0], 0)
    emit_loads(iters[0], 1)
    emit_exp(iters[0], 0)
    emit_bbuild(iters[0], 0)
    emit_front_a(iters[1])
    for idx, it in enumerate(iters):
        nxt = iters[idx + 1] if idx + 1 < len(iters) else None
        nxt2 = iters[idx + 2] if idx + 2 < len(iters) else None
        v = st[it]
        v["ys_h"] = {h: ph2.tile([96, S, T], BF16, tag=f"ysh{h}",
                                 name=f"ysh{h}") for h in (0, 1)}
        # ---- group 0 ----
        emit_carry_in(it, 0)
        emit_bbuild(it, 1, gate_src=True)
        emit_scan(it, 0)            # Pool bbuild(it,1) runs under this scan
        emit_exp(it, 1)
        emit_carry_out(it, 0)
        if nxt is not None:
            emit_front_b1(nxt)      # silu mults + xp/dt chain (front_a ran
                                    # during the previous iteration's g1)
        emit_reduce(it, 0)
        if idx in (0, 2, 4):
            emit_ln_block(idx // 2 + 1)     # deferred LN: consumers are
            emit_ln_block(6 - idx // 2)     # >= 2 iterations downstream
        if nxt is not None:
            emit_front_b2(nxt)      # du mult + DRAM staging
            emit_loads(nxt, 0)
        # ---- group 1 ----
        emit_carry_in(it, 1)
        if nxt2 is not None:
            emit_front_a(nxt2)      # PE/ACT front two iterations ahead
        emit_scan(it, 1)
        if nxt is not None:
            emit_exp(nxt, 0)
        emit_carry_out(it, 1)
        emit_reduce(it, 1)
        if nxt is not None:
            emit_bbuild(nxt, 0)     # loads(nxt,0) have long landed by now
            emit_loads(nxt, 1)
        emit_gating(it)
        if it[1] == "b" and it[0] >= NCH // 2:
            emit_phase3(it[0])
            emit_phase3(NCH - 1 - it[0])
        prev = iters[idx - 1] if idx else None
        if prev in st:
            del st[prev]

    ctx.close()


# ---------------- host side ----------------

def _prep_params(inputs):
    bf = ml_dtypes.bfloat16
    p = {}
    ln_w = inputs["ln_w"].astype(np.float64)
    assert np.abs(inputs["ln_b"]).max() == 0.0, "ln_b folding not implemented"
    for s_ in ("f", "b"):
        w = inputs[f"in_proj_w_{s_}"].astype(np.float64) * ln_w[None, :]
        wu = w[0:D_INNER]                       # [192, 96]
        cw = inputs[f"conv_w_{s_}"].astype(np.float64)  # [192, 4]
        taps = [np.ascontiguousarray(wu.T) * cw[:, k][None, :]
                for k in range(D_CONV)]         # each [96, 192]
        p[f"w_u_{s_}"] = np.concatenate(taps, axis=1).astype(bf)
        p[f"w_z_{s_}"] = np.ascontiguousarray(w[D_INNER:].T).astype(bf)
        xp = np.zeros((D_INNER, 80), np.float32)
        xpw = inputs[f"x_proj_w_{s_}"]          # [38, 192]
        xp[:, 0:DT_RANK] = xpw[0:DT_RANK].T
        xp[:, 32:32 + D_STATE] = xpw[DT_RANK:DT_RANK + D_STATE].T
        xp[:, 64:64 + D_STATE] = xpw[DT_RANK + D_STATE:].T
        p[f"w_xp_{s_}"] = xp.astype(bf)
        p[f"w_dtp_{s_}"] = np.ascontiguousarray(inputs[f"dt_proj_w_{s_}"].T).astype(bf)
        p[f"conv_b_{s_}"] = inputs[f"conv_b_{s_}"].reshape(D_INNER, 1).astype(np.float32)
        p[f"dt_bias_{s_}"] = inputs[f"dt_bias_{s_}"].reshape(D_INNER, 1).astype(np.float32)
        p[f"d_skip_{s_}"] = inputs[f"D_{s_}"].reshape(D_INNER, 1).astype(np.float32)
    p["w_out"] = np.ascontiguousarray(inputs["out_proj_w"].T).astype(bf)
    p["ident"] = np.eye(128, dtype=bf)
    a_f = np.exp(inputs["A_log_f"][0]).astype(np.float32)
    assert np.allclose(np.exp(inputs["A_log_f"]), np.tile(a_f, (D_INNER, 1)))
    assert np.allclose(np.exp(inputs["A_log_b"]), np.tile(a_f, (D_INNER, 1)))
    p["_a_vals"] = [float(v) for v in a_f]
    return p


def _pixel_shuffle(x):
    B, C, H, W = x.shape
    nh, nw = H // P_PIX, W // P_PIX
    xd = x.reshape(B, C, nh, P_PIX, nw, P_PIX).transpose(0, 3, 5, 1, 2, 4)
    return xd.reshape(B * P_PIX * P_PIX, C, nh * nw)


def _pixel_unshuffle(y):
    nh = nw = NH
    x = y.reshape(1, P_PIX, P_PIX, D_MODEL, nh, nw).transpose(0, 3, 4, 1, 5, 2)
    return np.ascontiguousarray(x.reshape(1, D_MODEL, HW_, HW_))


_COMPILED = {}


def _split_dma_waits(nc, max_waits=1):
    """The HW pseudo-DMA supports at most 2 sem waits; move the rest onto a
    preceding NoOp on the issuing engine (same semantics, program order)."""
    nid = [0]
    for f in nc.m.functions:
        for b in f.blocks:
            il = b.instructions
            out = []
            changed = False
            for inst in il:
                si = getattr(inst, "sync_info", None)
                if (type(inst).__name__ != "InstNoOp" and si is not None
                        and si.on_wait is not None and len(si.on_wait) > max_waits):
                    excess = list(si.on_wait[:-max_waits])
                    keep = list(si.on_wait[-max_waits:])
                    for w in excess:
                        nop = mybir.InstNoOp(
                            name=f"dmawait-nop-{nid[0]}", engine=inst.engine,
                            ins=[], outs=[],
                            sync_info=mybir.SyncInfo(on_wait=[w], on_update=[]))
                        nid[0] += 1
                        out.append(nop)
                    inst.sync_info = mybir.SyncInfo(
                        on_wait=keep, on_update=list(si.on_update or []))
                    changed = True
                out.append(inst)
            if changed:
                b.instructions = out


def _get_compiled(cfg, a_vals, engines=None, split_waits=True):
    key = (cfg.L, cfg.T, cfg.S, tuple(a_vals), str(engines), split_waits)
    if key not in _COMPILED:
        nc = bass.Bass("TRN2", target_bir_lowering=False, debug=False)
        with tile.TileContext(nc) as tc:
            build_kernel(nc, tc, cfg, a_vals, engines=engines)
        if split_waits:
            _split_dma_waits(nc)
        _COMPILED[key] = nc
    return _COMPILED[key]


COUNTS = [13, 13, 13, 13, 12, 12, 12, 12]


def make_in_maps(x, p, cfg):
    xs = _pixel_shuffle(x.astype(np.float32))
    in_maps = []
    off = 0
    S = cfg.S
    for ci in range(NCORES):
        cnt = COUNTS[ci]
        sl = xs[off:off + cnt]
        off += cnt
        if cnt < S:
            sl = np.concatenate([sl, np.zeros((S - cnt, D_MODEL, cfg.L), np.float32)], 0)
        m = {"xtok": np.ascontiguousarray(sl.transpose(0, 2, 1).reshape(cfg.TOK, D_MODEL)),
             "x_T": np.ascontiguousarray(sl.transpose(1, 0, 2).reshape(D_MODEL, cfg.TOK))}
        m.update(p)
        in_maps.append(m)
    return in_maps


def kernel(**inputs):
    inputs = {k: np.asarray(v) for k, v in inputs.items()}
    x = inputs["x"]
    cfg = Cfg()
    p = _prep_params(inputs)
    a_vals = p.pop("_a_vals")
    in_maps = make_in_maps(x, p, cfg)
    nc = _get_compiled(cfg, a_vals)
    res = run_bass_kernel_spmd(nc, in_maps, list(range(NCORES)))
    y = np.empty((NB, D_MODEL, L_FULL), np.float32)
    off = 0
    for ci in range(NCORES):
        o = np.asarray(res.results[ci]["out"]).reshape(D_MODEL, cfg.S, L_FULL)
        cnt = COUNTS[ci]
        y[off:off + cnt] = o.transpose(1, 0, 2)[:cnt]
        off += cnt
    return _pixel_unshuffle(y).astype(x.dtype)
